# revision 7
# baseline (speedup 1.0000x reference)
"""MoE layer (8 experts, top-2 routing + shared expert) on 8 Trainium2 cores.

Strategy (expert parallelism per the sharding hint):
  - Host computes the router (logits -> softmax -> top-2 -> combine weights)
    and *dispatches*: core e receives the tokens routed to expert e plus a
    1/8 data-parallel slice of all tokens for the shared expert.
  - Each core runs one Bass/Tile kernel computing, for its token set,
      y = (silu(x @ Wg.T) * (x @ Wu.T)) @ Wd.T   (scaled by combine weight)
    for its expert's weights, then the same with the shared-expert weights.
  - Host *combines*: scatter-adds per-expert outputs into the full [N, D].

Numerics: fp8e4m3 DoubleRow matmuls (0.5 cycles/row, 256-deep contraction)
with an exact two-term (hi+lo) operand split. Each 256-row contraction
slice j needs 3 products (hi*hi, hi*lo, lo*hi; the lo*lo term is ~7e-4 and
dropped), so the PE runs at 4/3 the fp16 rate while keeping the end-to-end
error ~4e-3 (verified against the fp32 reference).

Scaling: weights are pre-scaled on host so both hi and lo terms stay in
(or above) e4m3's subnormal range: Wg*32, Wu*8, Wd*32. The device keeps
activations in a "8*a" domain: silu16 = Silu(pg/32), t16 = silu16 * pu
(pu = 8u), a_hi = fp8(t16), a_lo = fp8(t16 - a_hi). The final 1/(8*32) is
folded into the per-token combine weight.

Layouts (per core, all fp8 except cw/y):
  xh/xl   [128, nch, 8, 2, 256]  (k-pair-major token chunks; tail chunk of
                                  128 tokens goes in separate xh_t/xl_t)
  wg,wu   [128, 11, 8, 2, 128]   hi/lo pairs, expert + shared
  wd      [128, 6, 2, 2048]      h-pair layout; jh=5 packs (h10,h10) for hi
                                  and (lo10, 0) for lo, so the odd 11th
                                  h-tile costs 2 instrs instead of 3
  aT      ath [128, 6, 2, 256] = (ahi pairs; slot [5,1] holds alo10)
          atl [128, 5, 2, 256] = alo pairs for jh<5
  cw      [128, TT/128] f32 (combine weight / 256), y [TT, 2048] fp16
"""

import numpy as np
import ml_dtypes

import concourse.mybir as mybir
import concourse.tile as tile
from concourse import bacc
from concourse.bass import ds
from concourse.bass_utils import run_bass_kernel_spmd

P = 128
D = 2048
H = 1408
E = 8
TOP_K = 2
KJ = D // 256      # 8 k-pairs
NHT = H // P       # 11 h-tiles
JH = 6             # h-pairs incl. packed tail
DT8 = mybir.dt.float8e4
F16 = mybir.dt.float16
F32 = mybir.dt.float32
NP8 = ml_dtypes.float8_e4m3
DR = mybir.MatmulPerfMode.DoubleRow

SWG, SWU, SWD = 32.0, 8.0, 32.0
OSCALE = 1.0 / (SWU * SWD)   # folded into cw on host


def _subchunks(count, base):
    """Split count tokens into 256-token subchunks (+ one 128 tail)."""
    out = []
    pos = 0
    while count - pos >= 256:
        out.append((base + pos, 256))
        pos += 256
    if count - pos >= P:
        out.append((base + pos, P))
        pos += P
    assert pos == count
    return out


def build_kernel(C, S0, repeat=1, delay=2, xb=4, ab=None, ob=3, pgb=2, pyb=4,
                 wsplit=3, dsplit=2):
    TT = C + S0
    assert C % P == 0 and S0 % 256 == 0
    ab = ab if ab is not None else delay + 1

    nc = bacc.Bacc(
        "TRN2",
        target_bir_lowering=False,
        debug=False,
        enable_asserts=False,
        num_devices=8,
    )

    # token chunk tables (per phase)
    subs_e = _subchunks(C, 0)
    subs_s = _subchunks(S0, C)
    nch_e = sum(1 for _, w in subs_e if w == 256)
    tail_e = len(subs_e) - nch_e
    nch_s = len(subs_s)

    xh = nc.dram_tensor("xh", [P, nch_e + nch_s, KJ, 2, 256], DT8,
                        kind="ExternalInput").ap()
    xl = nc.dram_tensor("xl", [P, nch_e + nch_s, KJ, 2, 256], DT8,
                        kind="ExternalInput").ap()
    if tail_e:
        xh_t = nc.dram_tensor("xh_t", [P, KJ, 2, P], DT8, kind="ExternalInput").ap()
        xl_t = nc.dram_tensor("xl_t", [P, KJ, 2, P], DT8, kind="ExternalInput").ap()

    wts = {}
    for pref in ("e", "s"):
        wts[pref] = {
            nm: nc.dram_tensor(f"{nm}_{pref}",
                               [P, NHT, KJ, 2, P] if nm[1] != "d" else [P, JH, 2, D],
                               DT8, kind="ExternalInput").ap()
            for nm in ("wgh", "wgl", "wuh", "wul", "wdh", "wdl")
        }
    cw = nc.dram_tensor("cw", [P, TT // P], F32, kind="ExternalInput").ap()
    y = nc.dram_tensor("y", [TT, D], F16, kind="ExternalOutput").ap()
    y_r = y.rearrange("(g p) d -> p g d", p=P)

    # global subchunk program: (pref, start, w, chunk_idx or None for tail)
    prog = []
    for pref, subs in (("e", subs_e), ("s", subs_s)):
        for ci, (start, w) in enumerate(subs):
            gi = (ci if pref == "e" else nch_e + ci) if w == 256 else None
            prog.append((pref, start, w, gi))
    prog = prog * repeat

    with tile.TileContext(nc) as tc:
        with (
            tc.tile_pool(name="wghp", bufs=1) as wghp,
            tc.tile_pool(name="wglp", bufs=1) as wglp,
            tc.tile_pool(name="wuhp", bufs=1) as wuhp,
            tc.tile_pool(name="wulp", bufs=1) as wulp,
            tc.tile_pool(name="wdhp", bufs=1) as wdhp,
            tc.tile_pool(name="wdlp", bufs=1) as wdlp,
            tc.tile_pool(name="xhp", bufs=xb) as xhp,
            tc.tile_pool(name="xlp", bufs=xb) as xlp,
            tc.tile_pool(name="athp", bufs=ab) as athp,
            tc.tile_pool(name="atlp", bufs=ab) as atlp,
            tc.tile_pool(name="silp", bufs=2) as silp,
            tc.tile_pool(name="t16p", bufs=2) as t16p,
            tc.tile_pool(name="op", bufs=ob) as opool,
            tc.tile_pool(name="cp", bufs=1) as cpool,
            tc.tile_pool(name="psg", bufs=pgb, space="PSUM") as psgp,
            tc.tile_pool(name="psu", bufs=pgb, space="PSUM") as psup,
            tc.tile_pool(name="psy", bufs=pyb, space="PSUM") as psyp,
        ):
            cw_sb = cpool.tile([P, TT // P], F32)
            nc.sync.dma_start(cw_sb[:], cw)

            cur_w = {}          # SBUF weight tiles of the current phase
            emitted_phase = [None]
            pend = []           # subchunks awaiting gemm2

            def emit_weights(pref):
                d = wts[pref]
                t = {}
                t["wgh"] = wghp.tile([P, NHT, KJ, 2, P], DT8, tag="wgh", name="wgh_sb")
                t["wgl"] = wglp.tile([P, NHT, KJ, 2, P], DT8, tag="wgl", name="wgl_sb")
                t["wuh"] = wuhp.tile([P, NHT, KJ, 2, P], DT8, tag="wuh", name="wuh_sb")
                t["wul"] = wulp.tile([P, NHT, KJ, 2, P], DT8, tag="wul", name="wul_sb")
                # interleave gemm1 pieces so early h-tiles of g AND u land first
                bounds = [NHT * i // wsplit for i in range(wsplit + 1)]
                for b0, b1 in zip(bounds[:-1], bounds[1:]):
                    for nm in ("wgh", "wgl", "wuh", "wul"):
                        nc.sync.dma_start(t[nm][:, b0:b1], d[nm][:, b0:b1])
                t["wdh"] = wdhp.tile([P, JH, 2, D], DT8, tag="wdh", name="wdh_sb")
                t["wdl"] = wdlp.tile([P, JH, 2, D], DT8, tag="wdl", name="wdl_sb")
                dbounds = [JH * i // dsplit for i in range(dsplit + 1)]
                for b0, b1 in zip(dbounds[:-1], dbounds[1:]):
                    nc.sync.dma_start(t["wdh"][:, b0:b1], d["wdh"][:, b0:b1])
                    nc.sync.dma_start(t["wdl"][:, b0:b1], d["wdl"][:, b0:b1])
                return t

            def emit_gemm1(pref, start, w, gi, wt):
                # x chunk DMA
                if gi is not None:
                    xh_sb = xhp.tile([P, KJ, 2, 256], DT8, tag="xh", name="xh_sb")
                    xl_sb = xlp.tile([P, KJ, 2, 256], DT8, tag="xl", name="xl_sb")
                    nc.sync.dma_start(xh_sb[:], xh[:, gi])
                    nc.sync.dma_start(xl_sb[:], xl[:, gi])
                else:
                    xh_sb = xhp.tile([P, KJ, 2, 256], DT8, tag="xh", name="xh_sb")
                    xl_sb = xlp.tile([P, KJ, 2, 256], DT8, tag="xl", name="xl_sb")
                    nc.sync.dma_start(xh_sb[:, :, :, :P], xh_t)
                    nc.sync.dma_start(xl_sb[:, :, :, :P], xl_t)

                ath = athp.tile([P, JH, 2, 256], DT8, tag="ath", name="ath")[:, :, :, :w]
                atl = atlp.tile([P, JH - 1, 2, 256], DT8, tag="atl", name="atl")[:, :, :, :w]

                for ht in range(NHT):
                    pg = psgp.tile([P, 256], F32, tag="pg", name="pg")[:, :w]
                    for j in range(KJ):
                        lg = wt["wgh"][:, ht, j]
                        nc.tensor.matmul(pg, lg, xh_sb[:, j, :, :w],
                                         start=(j == 0), stop=False, perf_mode=DR)
                        nc.tensor.matmul(pg, lg, xl_sb[:, j, :, :w],
                                         start=False, stop=False, perf_mode=DR)
                        nc.tensor.matmul(pg, wt["wgl"][:, ht, j], xh_sb[:, j, :, :w],
                                         start=False, stop=(j == KJ - 1), perf_mode=DR)
                    sil = silp.tile([P, 256], F16, tag="sil", name="sil")[:, :w]
                    nc.scalar.activation(sil, pg, mybir.ActivationFunctionType.Silu,
                                         scale=1.0 / SWG)
                    pu = psup.tile([P, 256], F32, tag="pu", name="pu")[:, :w]
                    for j in range(KJ):
                        lu = wt["wuh"][:, ht, j]
                        nc.tensor.matmul(pu, lu, xh_sb[:, j, :, :w],
                                         start=(j == 0), stop=False, perf_mode=DR)
                        nc.tensor.matmul(pu, lu, xl_sb[:, j, :, :w],
                                         start=False, stop=False, perf_mode=DR)
                        nc.tensor.matmul(pu, wt["wul"][:, ht, j], xh_sb[:, j, :, :w],
                                         start=False, stop=(j == KJ - 1), perf_mode=DR)
                    t16 = t16p.tile([P, 256], F16, tag="t16", name="t16")[:, :w]
                    nc.vector.tensor_tensor(t16, sil, pu, mybir.AluOpType.mult)
                    hi_dst = ath[:, ht // 2, ht % 2, :]
                    nc.scalar.activation(hi_dst, t16,
                                         mybir.ActivationFunctionType.Copy)
                    lo_dst = ath[:, 5, 1, :] if ht == 10 else atl[:, ht // 2, ht % 2, :]
                    nc.vector.tensor_tensor(lo_dst, t16, hi_dst,
                                            mybir.AluOpType.subtract)
                return ath, atl

            def emit_gemm2(start, w, wt, ath, atl):
                for tg in range(w // P):
                    gg = (start + tg * P) // P
                    out_sb = opool.tile([P, D], F16, tag="o", name="out_sb")
                    for db in range(8):
                        py = psyp.tile([P, 256], F32, tag="py", name="py")
                        dcol = ds(db * 256, 256)
                        for jh in range(JH - 1):
                            la = ath[:, jh, :, tg * P:(tg + 1) * P]
                            nc.tensor.matmul(py, la, wt["wdh"][:, jh, :, dcol],
                                             start=(jh == 0), stop=False, perf_mode=DR)
                            nc.tensor.matmul(py, la, wt["wdl"][:, jh, :, dcol],
                                             start=False, stop=False, perf_mode=DR)
                            nc.tensor.matmul(py, atl[:, jh, :, tg * P:(tg + 1) * P],
                                             wt["wdh"][:, jh, :, dcol],
                                             start=False, stop=False, perf_mode=DR)
                        # jh=5: ath slots hold (ahi10, alo10); wdh jh5 = (Wd10, Wd10),
                        # wdl jh5 = (Wdlo10, 0) -> 2 instrs cover all 3 products
                        la = ath[:, 5, :, tg * P:(tg + 1) * P]
                        nc.tensor.matmul(py, la, wt["wdh"][:, 5, :, dcol],
                                         start=False, stop=False, perf_mode=DR)
                        nc.tensor.matmul(py, la, wt["wdl"][:, 5, :, dcol],
                                         start=False, stop=True, perf_mode=DR)
                        nc.vector.tensor_scalar_mul(out_sb[:, db * 256:(db + 1) * 256],
                                                    py, cw_sb[:, gg:gg + 1])
                    nc.sync.dma_start(y_r[:, gg, :], out_sb[:])

            for pref, start, w, gi in prog:
                if emitted_phase[0] != pref:
                    # drain pending gemm2s: they read the previous phase's
                    # weight tiles, which the new phase's DMAs will overwrite
                    while pend:
                        emit_gemm2(*pend.pop(0))
                    cur_w = emit_weights(pref)
                    emitted_phase[0] = pref
                ath, atl = emit_gemm1(pref, start, w, gi, cur_w)
                pend.append((start, w, cur_w, ath, atl))
                if len(pend) > delay:
                    emit_gemm2(*pend.pop(0))
            while pend:
                emit_gemm2(*pend.pop(0))

    nc.compile()
    return nc


def _route(x_flat, gate_w, expert_bias):
    """Replicate the reference router in numpy (fp32)."""
    N = x_flat.shape[0]
    logits = x_flat @ gate_w.T                       # [N, E]
    m = logits.max(-1, keepdims=True)
    p = np.exp(logits - m)
    p /= p.sum(-1, keepdims=True)
    biased = logits + expert_bias
    rows = np.arange(N)
    i1 = biased.argmax(-1)
    b2 = biased.copy()
    b2[rows, i1] = -np.inf
    i2 = b2.argmax(-1)
    w1 = p[rows, i1]
    w2 = p[rows, i2]
    s = w1 + w2
    return i1, i2, w1 / s, w2 / s


def _q8(v):
    return v.astype(NP8)


def _split8(v, s):
    vs = (v * s).astype(np.float32)
    hi = vs.astype(NP8)
    lo = (vs - hi.astype(np.float32)).astype(NP8)
    return hi, lo


def _pack_g(w8):
    """[H, D] fp8 -> [128, 11, 8, 2, 128] (p, ht, j, i, m)."""
    a = np.ascontiguousarray(
        w8.reshape(NHT, P, KJ, 2, P).transpose(4, 0, 2, 3, 1))
    return a


def _pack_d(w8pad):
    """[D, 1536] fp8 (padded/packd H) -> [128, 6, 2, D] (p, jh, i, dcol)."""
    a = np.ascontiguousarray(
        w8pad.reshape(D, JH, 2, P).transpose(3, 1, 2, 0))
    return a


def _pack_wd(Wd):
    """Wd [D, H] fp32 -> (wdh, wdl) packed [128, 6, 2, D] with the jh=5 trick.

    Packed h axis is (jh, i, m). jh<5 slots hold natural h<1280 for both hi
    and lo. jh=5: hi slots = (Wdhi10, Wdhi10), lo slots = (Wdlo10, 0), so a
    stationary pair (ahi10, alo10) covers all 3 tile-10 products in 2 instrs.
    """
    hi, lo = _split8(Wd, SWD)                       # [D, H] fp8
    hif = np.zeros((D, JH * 2 * P), dtype=NP8)
    lof = np.zeros((D, JH * 2 * P), dtype=NP8)
    hif[:, :H] = hi
    lof[:, :H] = lo
    hif[:, H:H + P] = hi[:, 10 * P:]                # (5,1) dup of Wdhi10
    # lof[:, H:H+P] stays 0                         # (5,1) zero
    return _pack_d(hif), _pack_d(lof)


def _pack_x(x8, subs):
    """[TT, D] fp8 + subchunk table -> (chunks [P, nch, 8, 2, 256], tail or None)."""
    full = []
    tail = None
    for start, w in subs:
        blk = x8[start:start + w].reshape(w, KJ, 2, P).transpose(3, 1, 2, 0)
        if w == 256:
            full.append(blk)
        else:
            tail = np.ascontiguousarray(blk)
    arr = np.ascontiguousarray(np.stack(full, axis=1)) if full else None
    return arr, tail


def _prepare(inputs):
    x = np.asarray(inputs["x"], dtype=np.float32)
    B, S_, D_ = x.shape
    assert D_ == D
    x_flat = x.reshape(-1, D)
    N = x_flat.shape[0]
    S0 = N // 8

    i1, i2, w1, w2 = _route(
        x_flat,
        np.asarray(inputs["gate_w"], dtype=np.float32),
        np.asarray(inputs["expert_bias"], dtype=np.float32),
    )

    idx_lists, w_lists = [], []
    for e in range(E):
        m1 = i1 == e
        m2 = i2 == e
        idx = np.nonzero(m1 | m2)[0]
        w = np.where(m1[idx], w1[idx], w2[idx]).astype(np.float32)
        idx_lists.append(idx)
        w_lists.append(w)

    maxc = max(len(ix) for ix in idx_lists)
    C = ((maxc + P - 1) // P) * P
    TT = C + S0
    subs_all = _subchunks(C, 0) + _subchunks(S0, C)

    xhi = _q8(x_flat)
    xlo = (x_flat - xhi.astype(np.float32)).astype(NP8)

    Wg = np.asarray(inputs["Wg"], dtype=np.float32)
    Wu = np.asarray(inputs["Wu"], dtype=np.float32)
    Wd = np.asarray(inputs["Wd"], dtype=np.float32)

    shared = {}
    gh, gl = _split8(np.asarray(inputs["Ws_g"], np.float32), SWG)
    shared["wgh_s"], shared["wgl_s"] = _pack_g(gh), _pack_g(gl)
    uh, ul = _split8(np.asarray(inputs["Ws_u"], np.float32), SWU)
    shared["wuh_s"], shared["wul_s"] = _pack_g(uh), _pack_g(ul)
    shared["wdh_s"], shared["wdl_s"] = _pack_wd(np.asarray(inputs["Ws_d"], np.float32))

    in_maps = []
    idx_pad = np.empty((E, C), dtype=np.int64)
    for e in range(E):
        idx = idx_lists[e]
        pad = np.full(C - len(idx), N, dtype=np.int64)  # N -> dummy row
        idx_pad[e] = np.concatenate([idx, pad])
        gather_idx = np.concatenate([idx, np.zeros(C - len(idx), np.int64)])

        rows = np.concatenate([gather_idx, np.arange(e * S0, (e + 1) * S0)])
        xh_c, xh_t = _pack_x(xhi[rows], subs_all)
        xl_c, xl_t = _pack_x(xlo[rows], subs_all)

        cwv = np.full(TT, OSCALE, dtype=np.float32)
        cwv[:len(idx)] = w_lists[e] * OSCALE
        cwv[len(idx):C] = 0.0
        cwv = np.ascontiguousarray(cwv.reshape(TT // P, P).T)

        gh, gl = _split8(Wg[e], SWG)
        uh, ul = _split8(Wu[e], SWU)
        wdh, wdl = _pack_wd(Wd[e])
        m = {
            "xh": xh_c, "xl": xl_c,
            "wgh_e": _pack_g(gh), "wgl_e": _pack_g(gl),
            "wuh_e": _pack_g(uh), "wul_e": _pack_g(ul),
            "wdh_e": wdh, "wdl_e": wdl,
            "cw": cwv,
            **shared,
        }
        if xh_t is not None:
            m["xh_t"] = xh_t
            m["xl_t"] = xl_t
        in_maps.append(m)
    return x, in_maps, idx_pad, C, S0, N


def _combine(x_shape, results, idx_pad, C, S0, N):
    acc = np.zeros((N + 1, D), dtype=np.float32)
    for e in range(E):
        ye = results[e]["y"].astype(np.float32)
        acc[idx_pad[e]] += ye[:C]
        acc[e * S0:(e + 1) * S0] += ye[C:]
    return acc[:N].reshape(x_shape)


def kernel(**inputs) -> np.ndarray:
    x, in_maps, idx_pad, C, S0, N = _prepare(inputs)
    nc = build_kernel(C, S0)
    res = run_bass_kernel_spmd(nc, in_maps, core_ids=list(range(8)))
    return _combine(x.shape, [res.results[e] for e in range(E)], idx_pad, C, S0, N)
